# revision 1
# baseline (speedup 1.0000x reference)
"""DeepSeekV3-style MoE layer on 8 Trainium2 NeuronCores.

Sharding strategy (expert-parallel, host-orchestrated dispatch):
  - The router (tiny: T x H x E matmul + sigmoid + top-2, ~0.1% of FLOPs) is
    computed on host with jax-on-CPU, replicating the reference bit-exactly so
    routing decisions / tie-breaks match.
  - Core e receives the tokens routed to expert e (gathered + transposed +
    zero-padded to a shared capacity C), and expert e's weights with the
    per-expert scalar mean routing weight folded into the down projection.
  - The shared expert is data-parallel: core c processes tokens
    [c*256, (c+1)*256).
  - Host combine: scatter-add routed outputs, add shared outputs.

Device kernel: SwiGLU MLP with tokens on the matmul moving (free) dim and
hidden/intermediate dims on partitions.  Matmuls run as float32r (full-rate
fp32 mode on the PE, 1 cycle/row for free dim >= 256) giving ~2.5e-4 rel err
vs the fp32 reference; MOE_DTYPE=bf16/hybrid switch to bf16-class compute.
Expert/x tensors are cached in SBUF; the (large, per-core-identical) shared
expert weights stream through a small pool as m-chunk pairs, one contiguous
DMA each.  PSUM: 6 accumulation banks + 2 down-proj banks.
"""

import os

os.environ.setdefault("JAX_PLATFORMS", "axon,cpu")

import numpy as np

# Problem constants (hardcoded per spec nn_DeepSeekV3MoE_11269994184873).
H = 1024       # hidden size
I = 512        # moe intermediate size
E = 8          # routed experts == n cores
K = 2          # experts per token
SI = 1024      # shared expert intermediate
B, S = 2, 1024
T = B * S      # 2048 tokens
P = 128
N_CORES = 8
TS = T // 4        # shared-expert tokens per core (512): 4-way token split
SIH = SI // 2      # shared-expert intermediate half per core: 2-way SI split

_nc_cache: dict = {}
last_nc = None  # exposed for test harness (TimelineSim)


def _round_up(v, m):
    return ((v + m - 1) // m) * m


def _host_router(x, gate_w, lb_bias):
    """Replicate the reference router on CPU via jax (bit-exact scores/top-k)."""
    import jax
    import jax.numpy as jnp

    cpu = jax.devices("cpu")[0]
    with jax.default_device(cpu):
        xf = jnp.asarray(np.asarray(x, np.float32)).reshape(-1, H)
        logits = xf @ jnp.asarray(np.asarray(gate_w, np.float32)).T + jnp.asarray(
            np.asarray(lb_bias, np.float32)
        )
        scores = jax.nn.sigmoid(logits.astype(jnp.float32))
        topw, topi = jax.lax.top_k(scores, K)
        topw = (topw / (topw.sum(-1, keepdims=True) + 1e-8)).astype(jnp.float32)
        wmeans = []
        for e in range(E):
            m = topi == e
            cnt = m.sum()
            wmean = (topw * m).sum() / jnp.maximum(cnt, 1).astype(topw.dtype)
            wmeans.append(wmean)
        topi_np = np.asarray(topi)
        wmean_np = np.asarray(jnp.stack(wmeans), np.float32)
    return topi_np, wmean_np


def _build_bass(C, mode="f32r"):
    """Build the SPMD Bass program for capacity C (multiple of 64, >=256)."""
    from contextlib import ExitStack

    import concourse.bacc as bacc
    import concourse.mybir as mybir
    import concourse.tile as tile

    f32 = mybir.dt.float32
    f32r = mybir.dt.float32r
    bf16 = mybir.dt.bfloat16
    # DTI: dtype of gate/up operands (x, wg, wu, sg, su)
    # DTH: dtype of h and down-proj weights (wd, sd)
    DTI, DTH = {
        "f32r": (f32r, f32r),
        "bf16": (bf16, bf16),
        "hybrid": (bf16, f32r),
    }[mode]
    Silu = mybir.ActivationFunctionType.Silu

    nc = bacc.Bacc("TRN2", target_bir_lowering=False, debug=False,
                   num_devices=N_CORES)

    # DRAM I/O (per-core values, same shapes on every core)
    xe = nc.dram_tensor("xe", [H // P, P, C], DTI, kind="ExternalInput")
    wg = nc.dram_tensor("wg", [H // P, P, I], DTI, kind="ExternalInput")
    wu = nc.dram_tensor("wu", [H // P, P, I], DTI, kind="ExternalInput")
    wd = nc.dram_tensor("wd", [I // P, P, H], DTH, kind="ExternalInput")
    xs = nc.dram_tensor("xs", [H // P, P, TS], DTI, kind="ExternalInput")
    # shared weights streamed as m-chunk PAIRS: [m2, p, j, k, c] so each pair
    # is one contiguous DMA; each core holds only its SI-half slice
    sg = nc.dram_tensor("sg", [SIH // (2 * P), P, 2, H // P, P], DTI,
                        kind="ExternalInput")
    su = nc.dram_tensor("su", [SIH // (2 * P), P, 2, H // P, P], DTI,
                        kind="ExternalInput")
    sd = nc.dram_tensor("sd", [H // (2 * P), P, 2, SIH // P, P], DTH,
                        kind="ExternalInput")
    ye = nc.dram_tensor("ye", [H // P, P, C], f32, kind="ExternalOutput")
    zs = nc.dram_tensor("zs", [H // P, P, TS], f32, kind="ExternalOutput")

    KH = H // P    # 8 k-chunks for H contraction
    KI = I // P    # 4 k-chunks for I contraction
    KS = SIH // P  # 4 k-chunks for SI-half contraction

    # token tiles for the routed phase: balanced sizes, multiples of 64,
    # each >= 256 (fp32r needs free dim >= 256 for full rate)
    nt = max(1, -(-C // 512))
    units = C // 64
    a_tiles = []
    off = 0
    for i in range(nt):
        u = units // nt + (1 if i < units % nt else 0)
        a_tiles.append((off, u * 64))
        off += u * 64
    assert off == C and all(tn >= 256 or C < 256 for _, tn in a_tiles)
    max_tn = max(tn for _, tn in a_tiles)

    with tile.TileContext(nc) as tc:
        with ExitStack() as ctx:
            const = ctx.enter_context(tc.tile_pool(name="const", bufs=1))
            spool = ctx.enter_context(tc.tile_pool(name="stream", bufs=3))
            hpool = ctx.enter_context(tc.tile_pool(name="h", bufs=2))
            tpool = ctx.enter_context(tc.tile_pool(name="tmp", bufs=2))
            opool = ctx.enter_context(tc.tile_pool(name="out", bufs=3))
            # PSUM budget: 8 banks total = 6 "acc" + 2 "y" (shared across phases)
            psACC = ctx.enter_context(tc.tile_pool(name="psACC", bufs=5, space="PSUM"))
            psY = ctx.enter_context(tc.tile_pool(name="psY", bufs=3, space="PSUM"))

            # ---- static loads (per-k, interleaved so PE starts early) ----
            x_sb = const.tile([P, KH, C], DTI, tag="x_sb")
            wg_sb = const.tile([P, KH, I], DTI, tag="wg_sb")
            wu_sb = const.tile([P, KH, I], DTI, tag="wu_sb")
            for k in range(KH):
                nc.sync.dma_start(x_sb[:, k, :], xe[k])
                nc.sync.dma_start(wg_sb[:, k, :], wg[k])
                nc.sync.dma_start(wu_sb[:, k, :], wu[k])
            wd_sb = const.tile([P, KI, H], DTH, tag="wd_sb")
            for k in range(KI):
                nc.sync.dma_start(wd_sb[:, k, :], wd[k])
            xs_sb = const.tile([P, KH, TS], DTI, tag="xs_sb")
            nc.sync.dma_start(xs_sb[:], xs.ap().rearrange("k p t -> p k t"))

            # ---- interleaved emission: phase A token-tiles alternate with
            # phase B gate/up pairs so the (static) PE stream matches the DMA
            # arrival order ----
            h_s = const.tile([P, KS, TS], DTH, tag="h_s")
            npairs = SIH // (2 * P)

            def emit_phA_tile(off, tn):
                h_a = hpool.tile([P, KI, max_tn], DTH, tag="h_a", name=f"h_a{off}")
                xr = x_sb[:, :, off:off + tn]
                for m in range(I // P):
                    pg = psACC.tile([P, 512], f32, tag="acc", name=f"apg{off}_{m}")
                    pu = psACC.tile([P, 512], f32, tag="acc", name=f"apu{off}_{m}")
                    for k in range(KH):
                        nc.tensor.matmul(
                            pg[:, :tn],
                            wg_sb[:, k, m * P:(m + 1) * P],
                            xr[:, k, :],
                            start=(k == 0), stop=(k == KH - 1),
                        )
                    for k in range(KH):
                        nc.tensor.matmul(
                            pu[:, :tn],
                            wu_sb[:, k, m * P:(m + 1) * P],
                            xr[:, k, :],
                            start=(k == 0), stop=(k == KH - 1),
                        )
                    tg = tpool.tile([P, 512], f32, tag="tmp_silu",
                                    name=f"atg{off}_{m}")
                    nc.scalar.activation(tg[:, :tn], pg[:, :tn], Silu)
                    nc.vector.tensor_mul(h_a[:, m, :tn], tg[:, :tn], pu[:, :tn])
                y_sb = opool.tile([P, H // P, max_tn], f32, tag="y_sb",
                                  name=f"y_sb{off}")
                for m in range(H // P):
                    py = psY.tile([P, 512], f32, tag="y", name=f"apy{off}_{m}")
                    for k in range(KI):
                        nc.tensor.matmul(
                            py[:, :tn],
                            wd_sb[:, k, m * P:(m + 1) * P],
                            h_a[:, k, :tn],
                            start=(k == 0), stop=(k == KI - 1),
                        )
                    nc.any.tensor_copy(y_sb[:, m, :tn], py[:, :tn])
                nc.sync.dma_start(
                    ye.ap().rearrange("m p c -> p m c")[:, :, off:off + tn],
                    y_sb[:, :, :tn])

            _pf = {}

            def prefetch_phB_pair(m2):
                sgm = spool.tile([P, 2, KH, P], DTI, tag="sgm", name=f"sgm{m2}")
                nc.sync.dma_start(sgm[:], sg[m2])
                sum_ = spool.tile([P, 2, KH, P], DTI, tag="sum_", name=f"sum{m2}")
                nc.sync.dma_start(sum_[:], su[m2])
                _pf[m2] = (sgm, sum_)

            def emit_phB_pair(m2):
                if m2 in _pf:
                    sgm, sum_ = _pf.pop(m2)
                else:
                    sgm = spool.tile([P, 2, KH, P], DTI, tag="sgm",
                                     name=f"sgm{m2}")
                    nc.sync.dma_start(sgm[:], sg[m2])
                    sum_ = spool.tile([P, 2, KH, P], DTI, tag="sum_",
                                      name=f"sum{m2}")
                    nc.sync.dma_start(sum_[:], su[m2])
                for j in range(2):
                    m = 2 * m2 + j
                    pg = psACC.tile([P, 512], f32, tag="acc", name=f"bpg{m2}_{j}")
                    pu = psACC.tile([P, 512], f32, tag="acc", name=f"bpu{m2}_{j}")
                    for k in range(KH):
                        nc.tensor.matmul(
                            pg[:, :TS], sgm[:, j, k, :], xs_sb[:, k, :],
                            start=(k == 0), stop=(k == KH - 1),
                        )
                    for k in range(KH):
                        nc.tensor.matmul(
                            pu[:, :TS], sum_[:, j, k, :], xs_sb[:, k, :],
                            start=(k == 0), stop=(k == KH - 1),
                        )
                    ts_ = tpool.tile([P, 512], f32, tag="tmp_silu",
                                     name=f"bts{m2}_{j}")
                    nc.scalar.activation(ts_[:, :TS], pg[:, :TS], Silu)
                    nc.vector.tensor_mul(h_s[:, m, :], ts_[:, :TS], pu[:, :TS])

            for i in range(npairs):
                prefetch_phB_pair(i)
            _pfd = {}
            for i in range(len(a_tiles)):
                emit_phA_tile(*a_tiles[i])
            for i in range(npairs):
                emit_phB_pair(i)

            zre = zs.ap().rearrange("m p t -> p m t")
            for m2 in range(H // (2 * P)):
                if m2 in _pfd:
                    sdm = _pfd.pop(m2)
                else:
                    sdm = spool.tile([P, 2, KS, P], DTH, tag="sdm",
                                     name=f"sdm{m2}")
                    nc.sync.dma_start(sdm[:], sd[m2])
                z_sb = opool.tile([P, 2, TS], f32, tag="z_sb", name=f"z_sb{m2}")
                for j in range(2):
                    py = psY.tile([P, 512], f32, tag="y", name=f"bpy{m2}_{j}")
                    for k in range(KS):
                        nc.tensor.matmul(
                            py[:, :TS], sdm[:, j, k, :], h_s[:, k, :],
                            start=(k == 0), stop=(k == KS - 1),
                        )
                    nc.any.tensor_copy(z_sb[:, j, :], py[:, :TS])
                nc.sync.dma_start(zre[:, 2 * m2:2 * m2 + 2, :], z_sb[:])

    nc.finalize()
    return nc


DTYPE_MODE = os.environ.get("MOE_DTYPE", "f32r")


def _get_nc(C):
    global last_nc
    key = (C, DTYPE_MODE)
    if key not in _nc_cache:
        _nc_cache[key] = _build_bass(C, DTYPE_MODE)
    last_nc = _nc_cache[key]
    return _nc_cache[key]


def kernel(x, gate_w, lb_bias, expert_gate_w, expert_up_w, expert_down_w,
           shared_gate_w, shared_up_w, shared_down_w):
    from concourse.bass_utils import run_bass_kernel_spmd

    x = np.asarray(x, np.float32)
    gate_w = np.asarray(gate_w, np.float32)
    lb_bias = np.asarray(lb_bias, np.float32)
    egw = np.asarray(expert_gate_w, np.float32)
    euw = np.asarray(expert_up_w, np.float32)
    edw = np.asarray(expert_down_w, np.float32)
    sgw = np.asarray(shared_gate_w, np.float32)
    suw = np.asarray(shared_up_w, np.float32)
    sdw = np.asarray(shared_down_w, np.float32)

    xf = x.reshape(T, H)

    # ---- host router (replicates reference) ----
    topi, wmean = _host_router(x, gate_w, lb_bias)

    sel = [np.nonzero((topi == e).any(axis=-1))[0] for e in range(E)]
    counts = [len(s) for s in sel]
    C = max(_round_up(max(counts), 64), 256)

    nc = _get_nc(C)

    # ---- per-core inputs ----
    xfT = np.ascontiguousarray(xf.T)  # [H, T]

    # m-chunk-pair-major shared weights: [m2, p, j, k, c] with
    # lhsT[k*P+p, (2*m2+j)*P+c]
    def _pairs(wT, MD):
        # wT: [K_dim, MD] (already transposed weight)
        KD = wT.shape[0]
        a = wT.reshape(KD // P, P, MD // (2 * P), 2, P)   # [k, p, m2, j, c]
        return a.transpose(2, 1, 3, 0, 4)                 # [m2, p, j, k, c]

    # per-SI-half shared weights (core // 4 picks the half)
    sgT_h = [_pairs(sgw[h * SIH:(h + 1) * SIH].T, SIH) for h in range(2)]
    suT_h = [_pairs(suw[h * SIH:(h + 1) * SIH].T, SIH) for h in range(2)]
    sdT_h = [_pairs(np.ascontiguousarray(sdw[:, h * SIH:(h + 1) * SIH]).T, H)
             for h in range(2)]

    import ml_dtypes
    bfc = lambda a: np.ascontiguousarray(a).astype(ml_dtypes.bfloat16)
    f32c = lambda a: np.ascontiguousarray(a, np.float32)
    # cast_i: gate/up operands; cast_h: down-proj weights
    cast_i, cast_h = {
        "f32r": (f32c, f32c),
        "bf16": (bfc, bfc),
        "hybrid": (bfc, f32c),
    }[DTYPE_MODE]
    sgT_h = [cast_i(a) for a in sgT_h]
    suT_h = [cast_i(a) for a in suT_h]
    sdT_h = [cast_h(a) for a in sdT_h]
    in_maps = []
    for e in range(E):
        xe = np.zeros((H // P, P, C), np.float32)
        if counts[e]:
            xe.reshape(H, C)[:, :counts[e]] = xfT[:, sel[e]]
        wgT = cast_i(egw[e].T).reshape(H // P, P, I)
        wuT = cast_i(euw[e].T).reshape(H // P, P, I)
        wdT = cast_h((edw[e] * wmean[e]).T).reshape(I // P, P, H)
        tsl = e % 4    # token-slice index
        sh = e // 4    # SI half
        xs = cast_i(xfT[:, tsl * TS:(tsl + 1) * TS]).reshape(H // P, P, TS)
        in_maps.append({
            "xe": cast_i(xe), "wg": wgT, "wu": wuT, "wd": wdT,
            "xs": xs, "sg": sgT_h[sh], "su": suT_h[sh], "sd": sdT_h[sh],
        })

    res = run_bass_kernel_spmd(nc, in_maps, core_ids=list(range(N_CORES)))

    # ---- host combine ----
    out = np.zeros((T, H), np.float32)
    for e in range(E):
        if counts[e]:
            ye = res.results[e]["ye"].reshape(H, C)
            out[sel[e]] += ye[:, :counts[e]].T
        zsout = res.results[e]["zs"].reshape(H, TS)
        tsl = e % 4
        out[tsl * TS:(tsl + 1) * TS] += zsout.T
    return out.reshape(B, S, H).astype(x.dtype)



# revision 5
# speedup vs baseline: 1.2428x; 1.2428x over previous
"""DeepSeekV3-style MoE layer on 8 Trainium2 NeuronCores.

Sharding (expert-parallel, host-orchestrated dispatch):
  - Router (tiny) on host via jax-on-CPU, bit-exact with the reference.
  - Core e computes expert e over its routed tokens (padded to capacity C);
    the shared expert is split 4-way over tokens x 2-way over SI.
  - Host combine: scatter-add routed outputs + shared outputs.

Device kernel: all matmuls run as fp8e4m3 DoubleRow (2 k-chunks per
instruction, 0.5 cycles/row) with 3-term error compensation:
    W @ x  ~=  Wh@xh + Wlo@xh + Wh@xl
where xh = q8(x), xl = q8(x - xh), Wh = q8(s*W), Wlo = q8(s*W - Wh), all
accumulating in one PSUM group.  Weight pre-scales keep every fp8 operand in
e4m3's normal range (max 240): gate x32, up x16, down x(2048/16).  The up
pre-scale rides through silu(g)*u so h is stored as 16*h (max |h| ~ 94); the
down pre-scale is divided out on the host.  End-to-end relmax vs the fp32
reference ~= 3e-3 (validated on CPU with ml_dtypes), on par with bf16.

DMA: every tensor is fp8 (half of bf16 traffic), outputs bf16.  Inputs are
issued in the exact order the (statically scheduled) PE stream consumes
them; phase A gate terms are emitted hi-sweep / lo-sweep / xl-sweep
(m-interleaved) so the PE never waits on a DMA that was issued later.
"""

import os

os.environ.setdefault("JAX_PLATFORMS", "axon,cpu")

import numpy as np
import ml_dtypes

# Problem constants (hardcoded per spec nn_DeepSeekV3MoE_11269994184873).
H = 1024       # hidden size
I = 512        # moe intermediate size
E = 8          # routed experts == n cores
K = 2          # experts per token
SI = 1024      # shared expert intermediate
B, S = 2, 1024
T = B * S      # 2048 tokens
P = 128
N_CORES = 8
TS = T // 4        # shared-expert tokens per core (512): 4-way token split
SIH = SI // 2      # shared-expert intermediate half per core: 2-way SI split
KH = H // P        # 8 k-chunks over H
KI = I // P        # 4 k-chunks over I
KS = SIH // P      # 4 k-chunks over SI-half

WS_G = 32.0        # gate weight pre-scale
WS_U = 16.0        # up weight pre-scale (h stored as WS_U * h)
WS_D = 2048.0      # down output scale (host divides it out)

E4 = ml_dtypes.float8_e4m3

_nc_cache: dict = {}
last_nc = None  # exposed for test harness (TimelineSim)


def _round_up(v, m):
    return ((v + m - 1) // m) * m


def _host_router(x, gate_w, lb_bias):
    """Replicate the reference router on CPU via jax (bit-exact scores/top-k)."""
    import jax
    import jax.numpy as jnp

    cpu = jax.devices("cpu")[0]
    with jax.default_device(cpu):
        xf = jnp.asarray(np.asarray(x, np.float32)).reshape(-1, H)
        logits = xf @ jnp.asarray(np.asarray(gate_w, np.float32)).T + jnp.asarray(
            np.asarray(lb_bias, np.float32)
        )
        scores = jax.nn.sigmoid(logits.astype(jnp.float32))
        topw, topi = jax.lax.top_k(scores, K)
        topw = (topw / (topw.sum(-1, keepdims=True) + 1e-8)).astype(jnp.float32)
        wmeans = []
        for e in range(E):
            m = topi == e
            cnt = m.sum()
            wmean = (topw * m).sum() / jnp.maximum(cnt, 1).astype(topw.dtype)
            wmeans.append(wmean)
        topi_np = np.asarray(topi)
        wmean_np = np.asarray(jnp.stack(wmeans), np.float32)
    return topi_np, wmean_np


def _tok_tiles(total, maxt):
    """Split `total` tokens into balanced tiles (multiples of 64, <= maxt)."""
    nt = max(1, -(-total // maxt))
    units = total // 64
    tiles = []
    off = 0
    for i in range(nt):
        u = units // nt + (1 if i < units % nt else 0)
        tiles.append((off, u * 64))
        off += u * 64
    assert off == total
    return tiles


def _build_bass(C):
    """Build the SPMD Bass program for capacity C (multiple of 64)."""
    from contextlib import ExitStack

    import concourse.bacc as bacc
    import concourse.mybir as mybir
    import concourse.tile as tile

    f32 = mybir.dt.float32
    bf16 = mybir.dt.bfloat16
    fp8 = mybir.dt.float8e4
    DR = mybir.MatmulPerfMode.DoubleRow
    Silu = mybir.ActivationFunctionType.Silu
    Copy = mybir.ActivationFunctionType.Copy

    nc = bacc.Bacc("TRN2", target_bir_lowering=False, debug=False,
                   num_devices=N_CORES)

    # DRAM I/O (per-core values, same shapes on every core); [k, p, f] layout,
    # loaded into SBUF as [p, k, f]
    def din(name, shape):
        return nc.dram_tensor(name, shape, fp8, kind="ExternalInput")

    xh_d = din("xh", [KH, P, C])
    xl_d = din("xl", [KH, P, C])
    wgh_d = din("wgh", [KH, P, I])
    wgl_d = din("wgl", [KH, P, I])
    wuh_d = din("wuh", [KH, P, I])
    wul_d = din("wul", [KH, P, I])
    wdh_d = din("wdh", [KI, P, H])
    wdl_d = din("wdl", [KI, P, H])
    xsh_d = din("xsh", [KH, P, TS])
    xsl_d = din("xsl", [KH, P, TS])
    sgh_d = din("sgh", [KH, P, SIH])
    sgl_d = din("sgl", [KH, P, SIH])
    suh_d = din("suh", [KH, P, SIH])
    sul_d = din("sul", [KH, P, SIH])
    sdh_d = din("sdh", [KS, P, H])
    sdl_d = din("sdl", [KS, P, H])
    ye = nc.dram_tensor("ye", [H // P, P, C], bf16, kind="ExternalOutput")
    zs = nc.dram_tensor("zs", [H // P, P, TS], bf16, kind="ExternalOutput")

    a_tiles = _tok_tiles(C, 512)    # routed-phase token tiles
    b_tiles = _tok_tiles(TS, 256)   # shared-phase token tiles (2 x 256)

    with tile.TileContext(nc) as tc:
        with ExitStack() as ctx:
            const = ctx.enter_context(tc.tile_pool(name="const", bufs=1))
            hpool = ctx.enter_context(tc.tile_pool(name="h", bufs=len(a_tiles) * 2 + len(b_tiles) * 2))
            tpool = ctx.enter_context(tc.tile_pool(name="tg", bufs=3))
            mpool = ctx.enter_context(tc.tile_pool(name="h32", bufs=3))
            opool = ctx.enter_context(tc.tile_pool(name="out", bufs=2))
            psACC = ctx.enter_context(tc.tile_pool(name="psACC", bufs=5, space="PSUM"))
            psY = ctx.enter_context(tc.tile_pool(name="psY", bufs=3, space="PSUM"))

            def sbuf_in(name, dram, kdim, fdim):
                return const.tile([P, kdim, fdim], fp8, tag=name, name=name)

            xh_sb = sbuf_in("xh_sb", xh_d, KH, C)
            xl_sb = sbuf_in("xl_sb", xl_d, KH, C)
            wgh_sb = sbuf_in("wgh_sb", wgh_d, KH, I)
            wgl_sb = sbuf_in("wgl_sb", wgl_d, KH, I)
            wuh_sb = sbuf_in("wuh_sb", wuh_d, KH, I)
            wul_sb = sbuf_in("wul_sb", wul_d, KH, I)
            wdh_sb = sbuf_in("wdh_sb", wdh_d, KI, H)
            wdl_sb = sbuf_in("wdl_sb", wdl_d, KI, H)
            xsh_sb = sbuf_in("xsh_sb", xsh_d, KH, TS)
            xsl_sb = sbuf_in("xsl_sb", xsl_d, KH, TS)
            sgh_sb = sbuf_in("sgh_sb", sgh_d, KH, SIH)
            sgl_sb = sbuf_in("sgl_sb", sgl_d, KH, SIH)
            suh_sb = sbuf_in("suh_sb", suh_d, KH, SIH)
            sul_sb = sbuf_in("sul_sb", sul_d, KH, SIH)
            sdh_sb = sbuf_in("sdh_sb", sdh_d, KS, H)
            sdl_sb = sbuf_in("sdl_sb", sdl_d, KS, H)

            def load(sb, dram, tok_slice=None):
                src = dram.ap().rearrange("k p f -> p k f")
                if tok_slice is None:
                    nc.sync.dma_start(sb[:], src)
                else:
                    o, n = tok_slice
                    nc.sync.dma_start(sb[:, :, o:o + n], src[:, :, o:o + n])

            # ---- input DMAs, in PE-consumption order ----
            load(xh_sb, xh_d, a_tiles[0])
            load(wgh_sb, wgh_d)
            for tsl in a_tiles[1:]:
                load(xh_sb, xh_d, tsl)
            load(wgl_sb, wgl_d)
            for tsl in a_tiles:
                load(xl_sb, xl_d, tsl)
            load(wuh_sb, wuh_d)
            load(wul_sb, wul_d)
            load(wdh_sb, wdh_d)
            load(wdl_sb, wdl_d)
            load(xsh_sb, xsh_d)
            load(sgh_sb, sgh_d)
            load(sgl_sb, sgl_d)
            load(xsl_sb, xsl_d)
            load(suh_sb, suh_d)
            load(sul_sb, sul_d)
            load(sdh_sb, sdh_d)
            load(sdl_sb, sdl_d)

            def acc3(ps, tn, wh, wl, mxh, mxl, moff, nm, nk, uid):
                """3-term DR accumulation into ps[:, :tn] for m-blocks.

                Emitted as hi-sweep / lo-sweep / xl-sweep over m so the PE
                stream matches DMA arrival order.  ps is a list of nm psum
                tiles (allocated in the hi sweep).
                """
                terms = [(wh, mxh), (wl, mxh), (wh, mxl)]
                for ti, (w, mx) in enumerate(terms):
                    for m in range(nm):
                        for kk in range(nk // 2):
                            nc.tensor.matmul(
                                ps[m][:, :tn],
                                w[:, 2 * kk:2 * kk + 2, moff + m * P:moff + (m + 1) * P],
                                mx[:, 2 * kk:2 * kk + 2],
                                start=(ti == 0 and kk == 0),
                                stop=(ti == 2 and kk == nk // 2 - 1),
                                perf_mode=DR,
                            )

            def mlp_wave(tag, t, off, tn, nmi, xh_t, xl_t, wgh_t, wgl_t,
                         wuh_t, wul_t, hh_t, hl_t):
                """Gate+up+silu+h-split for one token tile."""
                xh_s = xh_t[:, :, off:off + tn]
                xl_s = xl_t[:, :, off:off + tn]
                # gate: hi sweep (allocs), lo sweep, xl sweep
                pgs = [psACC.tile([P, 512], f32, tag="acc", name=f"{tag}pg{t}_{m}")
                       for m in range(nmi)]
                acc3(pgs, tn, wgh_t, wgl_t, xh_s, xl_s, 0, nmi, KH, f"{tag}g{t}")
                # silu for all m (frees pg slots for pu)
                tgs = []
                for m in range(nmi):
                    tg = tpool.tile([P, 512], f32, tag="tg", name=f"{tag}tg{t}_{m}")
                    nc.scalar.activation(tg[:, :tn], pgs[m][:, :tn], Silu,
                                         scale=1.0 / WS_G)
                    tgs.append(tg)
                # up
                pus = [psACC.tile([P, 512], f32, tag="acc", name=f"{tag}pu{t}_{m}")
                       for m in range(nmi)]
                acc3(pus, tn, wuh_t, wul_t, xh_s, xl_s, 0, nmi, KH, f"{tag}u{t}")
                # h = silu(g) * (WS_U*u); split into fp8 hi + lo
                for m in range(nmi):
                    h32 = mpool.tile([P, 512], f32, tag="h32",
                                     name=f"{tag}h32{t}_{m}")
                    nc.vector.tensor_mul(h32[:, :tn], tgs[m][:, :tn],
                                         pus[m][:, :tn])
                    nc.gpsimd.tensor_copy(hh_t[:, m, :tn], h32[:, :tn])
                    nc.gpsimd.tensor_sub(hl_t[:, m, :tn], h32[:, :tn],
                                         hh_t[:, m, :tn])

            def down_wave(tag, t, off, tn, nk_h, wdh_t, wdl_t, hh_t, hl_t,
                          out_d, copy_engine):
                """Down-proj for one token tile + output copy + DMA."""
                y_sb = opool.tile([P, H // P, tn], bf16, tag=f"{tag}y",
                                  name=f"{tag}y{t}")
                for mo in range(H // P):
                    py = psY.tile([P, 512], f32, tag="y", name=f"{tag}py{t}_{mo}")
                    terms = [(wdh_t, hh_t), (wdl_t, hh_t), (wdh_t, hl_t)]
                    for ti, (w, hx) in enumerate(terms):
                        for kk in range(nk_h // 2):
                            nc.tensor.matmul(
                                py[:, :tn],
                                w[:, 2 * kk:2 * kk + 2, mo * P:(mo + 1) * P],
                                hx[:, 2 * kk:2 * kk + 2, :tn],
                                start=(ti == 0 and kk == 0),
                                stop=(ti == 2 and kk == nk_h // 2 - 1),
                                perf_mode=DR,
                            )
                    if copy_engine == "act":
                        nc.scalar.activation(y_sb[:, mo, :], py[:, :tn], Copy)
                    else:
                        nc.vector.tensor_copy(y_sb[:, mo, :], py[:, :tn])
                nc.sync.dma_start(
                    out_d.ap().rearrange("m p c -> p m c")[:, :, off:off + tn],
                    y_sb[:])

            # ---- phase A (routed expert) ----
            ha_tiles = []
            for t, (off, tn) in enumerate(a_tiles):
                hh_t = hpool.tile([P, KI, tn], fp8, tag="hh", name=f"Ahh{t}")
                hl_t = hpool.tile([P, KI, tn], fp8, tag="hl", name=f"Ahl{t}")
                ha_tiles.append((hh_t, hl_t))
                mlp_wave("A", t, off, tn, I // P, xh_sb, xl_sb, wgh_sb, wgl_sb,
                         wuh_sb, wul_sb, hh_t, hl_t)
            for t, (off, tn) in enumerate(a_tiles):
                hh_t, hl_t = ha_tiles[t]
                down_wave("A", t, off, tn, KI, wdh_sb, wdl_sb, hh_t, hl_t,
                          ye, "act")

            # ---- phase B (shared expert) ----
            hb_tiles = []
            for t, (off, tn) in enumerate(b_tiles):
                hh_t = hpool.tile([P, KS, tn], fp8, tag="shh", name=f"Bhh{t}")
                hl_t = hpool.tile([P, KS, tn], fp8, tag="shl", name=f"Bhl{t}")
                hb_tiles.append((hh_t, hl_t))
                mlp_wave("B", t, off, tn, SIH // P, xsh_sb, xsl_sb, sgh_sb,
                         sgl_sb, suh_sb, sul_sb, hh_t, hl_t)
            for t, (off, tn) in enumerate(b_tiles):
                hh_t, hl_t = hb_tiles[t]
                down_wave("B", t, off, tn, KS, sdh_sb, sdl_sb, hh_t, hl_t,
                          zs, "pool")

    nc.finalize()
    return nc


def _get_nc(C):
    global last_nc
    key = C
    if key not in _nc_cache:
        _nc_cache[key] = _build_bass(C)
    last_nc = _nc_cache[key]
    return _nc_cache[key]


def _q8(a):
    return np.asarray(a, np.float32).astype(E4)


def _split8(a):
    """fp8 hi/lo split: a ~= hi + lo with hi = q8(a)."""
    hi = _q8(a)
    lo = _q8(np.asarray(a, np.float32) - hi.astype(np.float32))
    return hi, lo


def _kpf(a, kdim):
    """[D, F] -> [kdim, P, F] contiguous."""
    D, F = a.shape
    assert D == kdim * P
    return np.ascontiguousarray(a.reshape(kdim, P, F))


def kernel(x, gate_w, lb_bias, expert_gate_w, expert_up_w, expert_down_w,
           shared_gate_w, shared_up_w, shared_down_w):
    from concourse.bass_utils import run_bass_kernel_spmd

    x = np.asarray(x, np.float32)
    egw = np.asarray(expert_gate_w, np.float32)
    euw = np.asarray(expert_up_w, np.float32)
    edw = np.asarray(expert_down_w, np.float32)
    sgw = np.asarray(shared_gate_w, np.float32)
    suw = np.asarray(shared_up_w, np.float32)
    sdw = np.asarray(shared_down_w, np.float32)

    xf = x.reshape(T, H)

    # ---- host router (replicates reference) ----
    topi, wmean = _host_router(x, np.asarray(gate_w, np.float32),
                               np.asarray(lb_bias, np.float32))

    sel = [np.nonzero((topi == e).any(axis=-1))[0] for e in range(E)]
    counts = [len(s) for s in sel]
    C = max(_round_up(max(counts), 64), 256)

    nc = _get_nc(C)

    # ---- fp8 splits (shared across cores) ----
    xfT = np.ascontiguousarray(xf.T)          # [H, T]
    xh_T, xl_T = _split8(xfT)                  # [H, T] fp8

    # shared weights per SI-half
    sh_w = []
    for hf in range(2):
        sg_p = sgw[hf * SIH:(hf + 1) * SIH].T * WS_G    # [H, SIH]
        su_p = suw[hf * SIH:(hf + 1) * SIH].T * WS_U
        sd_p = np.ascontiguousarray(sdw[:, hf * SIH:(hf + 1) * SIH]).T \
            * (WS_D / WS_U)                              # [SIH, H]
        sgh, sgl = _split8(sg_p)
        suh, sul = _split8(su_p)
        sdh, sdl = _split8(sd_p)
        sh_w.append({
            "sgh": _kpf(sgh, KH), "sgl": _kpf(sgl, KH),
            "suh": _kpf(suh, KH), "sul": _kpf(sul, KH),
            "sdh": _kpf(sdh, KS), "sdl": _kpf(sdl, KS),
        })

    in_maps = []
    for e in range(E):
        cnt = counts[e]
        xh_e = np.zeros((KH, P, C), E4)
        xl_e = np.zeros((KH, P, C), E4)
        if cnt:
            xh_e.reshape(H, C)[:, :cnt] = xh_T[:, sel[e]]
            xl_e.reshape(H, C)[:, :cnt] = xl_T[:, sel[e]]
        wg_p = egw[e].T * WS_G                           # [H, I]
        wu_p = euw[e].T * WS_U
        wd_p = (edw[e] * (wmean[e] * WS_D / WS_U)).T     # [I, H]
        wgh, wgl = _split8(wg_p)
        wuh, wul = _split8(wu_p)
        wdh, wdl = _split8(wd_p)
        tsl = e % 4    # token-slice index
        shf = e // 4   # SI half
        m = {
            "xh": xh_e, "xl": xl_e,
            "wgh": _kpf(wgh, KH), "wgl": _kpf(wgl, KH),
            "wuh": _kpf(wuh, KH), "wul": _kpf(wul, KH),
            "wdh": _kpf(wdh, KI), "wdl": _kpf(wdl, KI),
            "xsh": np.ascontiguousarray(
                xh_T[:, tsl * TS:(tsl + 1) * TS]).reshape(KH, P, TS),
            "xsl": np.ascontiguousarray(
                xl_T[:, tsl * TS:(tsl + 1) * TS]).reshape(KH, P, TS),
        }
        m.update(sh_w[shf])
        in_maps.append(m)

    res = run_bass_kernel_spmd(nc, in_maps, core_ids=list(range(N_CORES)))

    # ---- host combine ----
    out = np.zeros((T, H), np.float32)
    inv = 1.0 / WS_D
    for e in range(E):
        if counts[e]:
            yee = res.results[e]["ye"].reshape(H, C).astype(np.float32)
            out[sel[e]] += yee[:, :counts[e]].T * inv
        zso = res.results[e]["zs"].reshape(H, TS).astype(np.float32)
        tsl = e % 4
        out[tsl * TS:(tsl + 1) * TS] += zso.T * inv
    return out.reshape(B, S, H).astype(x.dtype)
